# revision 29
# baseline (speedup 1.0000x reference)
"""Trainium2 Bass kernel for nn_A2aSparseMLP (GPT-OSS style top-2-of-8 MoE).

Strategy (expert-parallel, per the sharding hint):
  - Router runs on host (it is ~0.03% of total FLOPs) and produces the
    top-2 expert assignment + softmax weights for each of the 2048 tokens.
  - Tokens are dispatched (gathered) per expert on host; core e owns expert
    e's weights and computes the clamped-SwiGLU FFN for the tokens routed to
    it. Capacity is fixed at C=512 tokens per expert (the mean load is
    2048*2/8 = 512); the few tokens beyond 512 on an over-subscribed expert
    are computed exactly on host (~1-2% of the work for near-uniform routing).
  - Device math per core, all matmuls in bf16 with fp32 PSUM accumulation:
        guT[2I, C] = W_gu'^T @ x^T          (phase 1, transposed activations)
        act[I, C]  = (clamp(u)+1) * g * sigmoid(1.702 g)   (g = min(gate, 7))
        y[C, H]    = act^T.T @ W_down       (phase 2, natural row layout out)
    The transposed phase-1 layout means phase 2 consumes act directly as the
    stationary (lhsT) operand: no on-device transposes anywhere.
    The up-projection weights/bias are pre-scaled by 1/1.702 on host so the
    SwiGLU can use the ScalarEngine's Silu LUT:
        glu*(u+1) = silu(1.702*g) * ((u+1)/1.702)
  - Host combines: out[t] = sum_k w_k * (y_k[t] + down_bias[e_k]).

Weights are pre-swizzled on host into layouts that make every DMA fully
contiguous on both the HBM and SBUF side.
"""

import numpy as np
import ml_dtypes

BF16 = ml_dtypes.bfloat16

B, S, H, I, E = 1, 2048, 1024, 1024, 8
TOP_K = 2
ALPHA = 1.702
LIMIT = 7.0

P = 128
C = 512              # fixed per-expert token capacity on device
KT = H // P          # 8 k-tiles over the H (contraction) axis of phase 1
MT = 2 * I // P      # 16 m-tiles over the 2I axis (pair-interleaved on device)
IT = I // P          # 8 k-tiles over the I axis of phase 2
NH = H // 512        # 2 n-slices of the H output axis
N_WARM = 4           # PE warm-up matmuls to lift the HAM clock gate early
# device-side m order: gate j at slot 2j, up j at slot 2j+1 (pair-contiguous
# so each pair's weights arrive in one contiguous DMA)
M_SEQ = [m for j in range(IT) for m in (j, j + IT)]

_GRAPH = None


def _build_graph():
    import concourse.tile as tile
    from concourse import bacc, mybir

    f32 = mybir.dt.float32
    bf16 = mybir.dt.bfloat16
    ALU = mybir.AluOpType
    AF = mybir.ActivationFunctionType

    nc = bacc.Bacc("TRN2", target_bir_lowering=False, debug=False, num_devices=E)

    # xT grouped in 4 chunks of 2 k-tiles; wgu pair-contiguous per M_SEQ
    xT = nc.dram_tensor("xT", [P, KT, C], bf16, kind="ExternalInput")
    wgu = nc.dram_tensor("wgu", [IT, P, 2 * KT * P], bf16, kind="ExternalInput")
    wdn = nc.dram_tensor("wdn", [P, NH * IT * 512], bf16, kind="ExternalInput")
    bgu = nc.dram_tensor("bgu", [P, MT], f32, kind="ExternalInput")
    y = nc.dram_tensor("y", [C, H], bf16, kind="ExternalOutput")

    with tile.TileContext(nc) as tc:
        with (
            tc.tile_pool(name="persist", bufs=1) as persist,
            tc.tile_pool(name="work", bufs=4) as work,
            tc.tile_pool(name="yout", bufs=2) as yout,
            tc.tile_pool(name="ps_gu", bufs=5, space="PSUM") as ps_gu,
            tc.tile_pool(name="ps_y", bufs=2, space="PSUM") as ps_y,
            tc.tile_pool(name="ps_w", bufs=1, space="PSUM") as ps_w,
        ):
            # ---- PE warm-up: defeat the HAM clock gate while DMAs land ----
            warm_src = persist.tile([P, 512], bf16, tag="warm")
            nc.vector.memset(warm_src[:], 0.0)
            warm_ps = ps_w.tile([P, 512], f32, tag="warm_ps")
            for _ in range(N_WARM):
                nc.tensor.matmul(
                    warm_ps[:], warm_src[:, :P], warm_src[:], start=True, stop=True
                )

            # wg slot layout [pair, 2, KT*P]: slot (j,0)=gate j, (j,1)=up j
            wg_sb = persist.tile([P, IT, 2, KT * P], bf16, tag="wg")
            x_sb = persist.tile([P, KT, C], bf16, tag="x")

            # DMA order: x chunk 0, pair-0 weights, rest of x, bias, remaining
            # pairs. All on the sync HWDGE queue (other engines' dma_start
            # goes through SWDGE and is much slower to issue).
            nc.sync.dma_start(x_sb[:, 0:2, :], xT[:, 0:2, :])
            nc.sync.dma_start(wg_sb[:, 0, :, :], wgu[0].rearrange("p (t f) -> p t f", t=2))
            for kc in range(1, KT // 2):
                nc.sync.dma_start(
                    x_sb[:, 2 * kc : 2 * kc + 2, :], xT[:, 2 * kc : 2 * kc + 2, :]
                )
            b_sb = persist.tile([P, MT], f32, tag="b")
            nc.sync.dma_start(b_sb[:], bgu[:])
            for j in range(1, IT):
                nc.sync.dma_start(
                    wg_sb[:, j, :, :], wgu[j].rearrange("p (t f) -> p t f", t=2)
                )

            wd_sb = persist.tile([P, NH, IT * 512], bf16, tag="wd")
            nc.sync.dma_start(wd_sb[:], wdn.ap().rearrange("p (n f) -> p n f", n=NH))

            act_sb = persist.tile([P, IT, C], bf16, tag="act")

            # ---- phase 1 + activation chain (pair j = gate j, up j) ----
            for j in range(IT):
                psg = ps_gu.tile([P, C], f32, tag="gu")
                for k in range(KT):
                    nc.tensor.matmul(
                        psg[:],
                        wg_sb[:, j, 0, k * P : (k + 1) * P],
                        x_sb[:, k, :],
                        start=(k == 0),
                        stop=(k == KT - 1),
                    )
                psu = ps_gu.tile([P, C], f32, tag="gu")
                for k in range(KT):
                    nc.tensor.matmul(
                        psu[:],
                        wg_sb[:, j, 1, k * P : (k + 1) * P],
                        x_sb[:, k, :],
                        start=(k == 0),
                        stop=(k == KT - 1),
                    )
                # glu' = silu(ALPHA*gate + ALPHA*bg) straight from PSUM on the
                # ScalarEngine (gate bias pre-scaled by ALPHA on host).
                # The gate clamp commutes with silu: min(x,L)*sig(A*min(x,L))
                # == min(x*sig(A*x), L*sig(A*L)) since x*sig(A*x) is monotone,
                # so it is applied afterwards, fused into the final multiply.
                glu = work.tile([P, C], bf16, tag="glu")
                nc.scalar.activation(
                    glu[:], psg[:], AF.Silu,
                    bias=b_sb[:, 2 * j : 2 * j + 1], scale=ALPHA,
                )
                # up path is pre-scaled by 1/ALPHA on host:
                # u1 = min(up' + bu', LIMIT/ALPHA); u2 = max(u1, -LIMIT/ALPHA) + 1/ALPHA
                u2 = work.tile([P, C], bf16, tag="u2")
                nc.vector.tensor_scalar(
                    u2[:], psu[:], b_sb[:, 2 * j + 1 : 2 * j + 2], LIMIT / ALPHA,
                    ALU.add, ALU.min,
                )
                nc.vector.tensor_scalar(
                    u2[:], u2[:], -LIMIT / ALPHA, 1.0 / ALPHA, ALU.max, ALU.add
                )
                # act = min(glu', GLUMAX') * u2  ( = (u+1) * min(g,L) * sig(A*min(g,L)) )
                nc.vector.scalar_tensor_tensor(
                    act_sb[:, j, :], glu[:], GLUMAX_S, u2[:], ALU.min, ALU.mult
                )

            # ---- phase 2: y = act^T.T @ W_down ----
            for tm in range(C // P):
                y_t = yout.tile([P, H], bf16, tag="y_sb")
                for n in range(NH):
                    psy = ps_y.tile([P, 512], f32, tag="y")
                    for k in range(IT):
                        nc.tensor.matmul(
                            psy[:],
                            act_sb[:, k, tm * P : (tm + 1) * P],
                            wd_sb[:, n, k * 512 : (k + 1) * 512],
                            start=(k == 0),
                            stop=(k == IT - 1),
                        )
                    nc.vector.tensor_copy(y_t[:, n * 512 : (n + 1) * 512], psy[:])
                nc.sync.dma_start(y[tm * P : (tm + 1) * P, :], y_t[:])

    nc.compile()
    return nc


def get_graph():
    global _GRAPH
    if _GRAPH is None:
        _GRAPH = _build_graph()
    return _GRAPH


def _route(hs, router_w, router_b):
    """Host router: top-2 experts + softmax weights per token (float64)."""
    logits = hs.astype(np.float64) @ router_w.astype(np.float64) + router_b.astype(
        np.float64
    )
    top_idx = np.argsort(-logits, axis=1, kind="stable")[:, :TOP_K]  # [S, K]
    top_vals = np.take_along_axis(logits, top_idx, axis=1)
    ex = np.exp(top_vals - top_vals.max(axis=1, keepdims=True))
    topk_w = ex / ex.sum(axis=1, keepdims=True)  # [S, K]
    scores = np.zeros((hs.shape[0], E), np.float32)
    np.put_along_axis(scores, top_idx, topk_w.astype(np.float32), axis=1)
    return top_idx, topk_w.astype(np.float32), scores


def _dispatch(hs, top_idx, topk_w):
    """Token lists per expert, split into device (first C) and host overflow."""
    dev_idx, dev_w, over_idx, over_w = [], [], [], []
    for e in range(E):
        t, k = np.nonzero(top_idx == e)
        w = topk_w[t, k]
        dev_idx.append(t[:C])
        dev_w.append(w[:C])
        over_idx.append(t[C:])
        over_w.append(w[C:])
    return dev_idx, dev_w, over_idx, over_w


def make_in_maps(hs, gate_up_proj, gate_up_bias, down_proj, dev_idx):
    in_maps = []
    for e in range(E):
        n_e = len(dev_idx[e])
        xt = np.zeros((C, H), np.float32)
        xt[:n_e] = hs[dev_idx[e]]
        xt = np.ascontiguousarray(xt.reshape(C, KT, P).transpose(2, 1, 0)).astype(BF16)

        wg = gate_up_proj[e]  # [H, 2I] interleaved
        # de-interleave to [gate | up]; pre-scale the up half by 1/ALPHA
        wp = np.concatenate([wg[:, 0::2], wg[:, 1::2] * (1.0 / ALPHA)], axis=1)
        wp = (
            wp.reshape(KT, P, MT, P)
            .transpose(2, 1, 0, 3)
            .reshape(MT, P, KT * P)
        )
        # pair-contiguous: wgu[j] = [gate j | up j] k-major blocks
        wp = np.ascontiguousarray(
            np.stack([np.concatenate([wp[j], wp[j + IT]], axis=-1) for j in range(IT)])
        ).astype(BF16)

        wd = np.ascontiguousarray(
            down_proj[e]
            .reshape(IT, P, NH, 512)
            .transpose(1, 2, 0, 3)
            .reshape(P, NH * IT * 512)
        ).astype(BF16)

        bg = gate_up_bias[e]
        bp = np.concatenate([bg[0::2] * ALPHA, bg[1::2] * (1.0 / ALPHA)])
        bp = np.ascontiguousarray(bp.reshape(MT, P).T[:, M_SEQ]).astype(np.float32)

        in_maps.append(
            {
                "xT": np.ascontiguousarray(xt.reshape(P, KT, C)),
                "wgu": np.ascontiguousarray(wp),
                "wdn": np.ascontiguousarray(wd),
                "bgu": bp,
            }
        )
    return in_maps


def _expert_ffn_host(x, wgu_e, bgu_e, wdn_e):
    """Exact fp32 reference math for overflow tokens (no down bias)."""
    gu = x @ wgu_e + bgu_e
    gate = np.minimum(gu[:, 0::2], LIMIT)
    up = np.clip(gu[:, 1::2], -LIMIT, LIMIT)
    with np.errstate(over="ignore"):
        glu = gate / (1.0 + np.exp(np.minimum(-ALPHA * gate, 80.0)))
    return ((up + 1.0) * glu) @ wdn_e


def kernel(
    hidden_states,
    router_w,
    router_b,
    gate_up_proj,
    gate_up_bias,
    down_proj,
    down_bias,
):
    from concourse.bass_utils import run_bass_kernel_spmd

    hs = np.asarray(hidden_states, dtype=np.float32).reshape(S, H)
    router_w = np.asarray(router_w, dtype=np.float32)
    router_b = np.asarray(router_b, dtype=np.float32)
    gate_up_proj = np.asarray(gate_up_proj, dtype=np.float32)
    gate_up_bias = np.asarray(gate_up_bias, dtype=np.float32)
    down_proj = np.asarray(down_proj, dtype=np.float32)
    down_bias = np.asarray(down_bias, dtype=np.float32)

    top_idx, topk_w, scores = _route(hs, router_w, router_b)
    dev_idx, dev_w, over_idx, over_w = _dispatch(hs, top_idx, topk_w)

    nc = get_graph()
    in_maps = make_in_maps(hs, gate_up_proj, gate_up_bias, down_proj, dev_idx)
    res = run_bass_kernel_spmd(nc, in_maps, core_ids=list(range(E)))

    out = np.zeros((S, H), np.float32)
    for e in range(E):
        n_e = len(dev_idx[e])
        if n_e:
            ye = np.asarray(res.results[e]["y"][:n_e], dtype=np.float32) + down_bias[e][None, :]
            out[dev_idx[e]] += dev_w[e][:, None] * ye
        if len(over_idx[e]):
            yo = _expert_ffn_host(
                hs[over_idx[e]], gate_up_proj[e], gate_up_bias[e], down_proj[e]
            ) + down_bias[e][None, :]
            out[over_idx[e]] += over_w[e][:, None] * yo

    return out.reshape(B, S, H), scores.reshape(B, S, E)


# revision 30
# speedup vs baseline: 1.0045x; 1.0045x over previous
"""Trainium2 Bass kernel for nn_A2aSparseMLP (GPT-OSS style top-2-of-8 MoE).

Strategy (expert-parallel, per the sharding hint):
  - Router runs on host (it is ~0.03% of total FLOPs) and produces the
    top-2 expert assignment + softmax weights for each of the 2048 tokens.
  - Tokens are dispatched (gathered) per expert on host; core e owns expert
    e's weights and computes the clamped-SwiGLU FFN for the tokens routed to
    it. Capacity is fixed at C=512 tokens per expert (the mean load is
    2048*2/8 = 512); the few tokens beyond 512 on an over-subscribed expert
    are computed exactly on host (~1-2% of the work for near-uniform routing).
  - Device math per core, all matmuls in bf16 with fp32 PSUM accumulation:
        guT[2I, C] = W_gu'^T @ x^T          (phase 1, transposed activations)
        act[I, C]  = (clamp(u)+1) * g * sigmoid(1.702 g)   (g = min(gate, 7))
        y[C, H]    = act^T.T @ W_down       (phase 2, natural row layout out)
    The transposed phase-1 layout means phase 2 consumes act directly as the
    stationary (lhsT) operand: no on-device transposes anywhere.
    The up-projection weights/bias are pre-scaled by 1/1.702 on host so the
    SwiGLU can use the ScalarEngine's Silu LUT:
        glu*(u+1) = silu(1.702*g) * ((u+1)/1.702)
  - Host combines: out[t] = sum_k w_k * (y_k[t] + down_bias[e_k]).

Weights are pre-swizzled on host into layouts that make every DMA fully
contiguous on both the HBM and SBUF side.
"""

import numpy as np
import ml_dtypes

BF16 = ml_dtypes.bfloat16

B, S, H, I, E = 1, 2048, 1024, 1024, 8
TOP_K = 2
ALPHA = 1.702
LIMIT = 7.0

P = 128
C = 512              # fixed per-expert token capacity on device
KT = H // P          # 8 k-tiles over the H (contraction) axis of phase 1
MT = 2 * I // P      # 16 m-tiles over the 2I axis (pair-interleaved on device)
IT = I // P          # 8 k-tiles over the I axis of phase 2
NH = H // 512        # 2 n-slices of the H output axis
N_WARM = 4           # PE warm-up matmuls to lift the HAM clock gate early
# device-side m order: gate j at slot 2j, up j at slot 2j+1 (pair-contiguous
# so each pair's weights arrive in one contiguous DMA)
M_SEQ = [m for j in range(IT) for m in (j, j + IT)]

_GRAPH = None


def _build_graph():
    import concourse.tile as tile
    from concourse import bacc, mybir

    f32 = mybir.dt.float32
    bf16 = mybir.dt.bfloat16
    ALU = mybir.AluOpType
    AF = mybir.ActivationFunctionType

    nc = bacc.Bacc("TRN2", target_bir_lowering=False, debug=False, num_devices=E)

    # xT grouped in 4 chunks of 2 k-tiles; wgu pair-contiguous per M_SEQ
    xT = nc.dram_tensor("xT", [P, KT, C], bf16, kind="ExternalInput")
    wgu = nc.dram_tensor("wgu", [IT, P, 2 * KT * P], bf16, kind="ExternalInput")
    wdn = nc.dram_tensor("wdn", [P, NH * IT * 512], bf16, kind="ExternalInput")
    bgu = nc.dram_tensor("bgu", [P, MT], f32, kind="ExternalInput")
    y = nc.dram_tensor("y", [C, H], bf16, kind="ExternalOutput")

    with tile.TileContext(nc) as tc:
        with (
            tc.tile_pool(name="persist", bufs=1) as persist,
            tc.tile_pool(name="work", bufs=4) as work,
            tc.tile_pool(name="yout", bufs=3) as yout,
            tc.tile_pool(name="ps_gu", bufs=5, space="PSUM") as ps_gu,
            tc.tile_pool(name="ps_y", bufs=2, space="PSUM") as ps_y,
            tc.tile_pool(name="ps_w", bufs=1, space="PSUM") as ps_w,
        ):
            # ---- PE warm-up: defeat the HAM clock gate while DMAs land ----
            warm_src = persist.tile([P, 512], bf16, tag="warm")
            nc.vector.memset(warm_src[:], 0.0)
            warm_ps = ps_w.tile([P, 512], f32, tag="warm_ps")
            for _ in range(N_WARM):
                nc.tensor.matmul(
                    warm_ps[:], warm_src[:, :P], warm_src[:], start=True, stop=True
                )

            # wg slot layout [pair, 2, KT*P]: slot (j,0)=gate j, (j,1)=up j
            wg_sb = persist.tile([P, IT, 2, KT * P], bf16, tag="wg")
            x_sb = persist.tile([P, KT, C], bf16, tag="x")

            # DMA order: x chunk 0, pair-0 weights, rest of x, bias, remaining
            # pairs. All on the sync HWDGE queue (other engines' dma_start
            # goes through SWDGE and is much slower to issue).
            nc.sync.dma_start(x_sb[:, 0:2, :], xT[:, 0:2, :])
            nc.sync.dma_start(wg_sb[:, 0, :, :], wgu[0].rearrange("p (t f) -> p t f", t=2))
            for kc in range(1, KT // 2):
                nc.sync.dma_start(
                    x_sb[:, 2 * kc : 2 * kc + 2, :], xT[:, 2 * kc : 2 * kc + 2, :]
                )
            b_sb = persist.tile([P, MT], f32, tag="b")
            nc.sync.dma_start(b_sb[:], bgu[:])
            for j in range(1, IT):
                nc.sync.dma_start(
                    wg_sb[:, j, :, :], wgu[j].rearrange("p (t f) -> p t f", t=2)
                )

            wd_sb = persist.tile([P, NH, IT * 512], bf16, tag="wd")
            nc.sync.dma_start(wd_sb[:], wdn.ap().rearrange("p (n f) -> p n f", n=NH))

            act_sb = persist.tile([P, IT, C], bf16, tag="act")

            # ---- phase 1 + activation chain (pair j = gate j, up j) ----
            for j in range(IT):
                psg = ps_gu.tile([P, C], f32, tag="gu")
                for k in range(KT):
                    nc.tensor.matmul(
                        psg[:],
                        wg_sb[:, j, 0, k * P : (k + 1) * P],
                        x_sb[:, k, :],
                        start=(k == 0),
                        stop=(k == KT - 1),
                    )
                psu = ps_gu.tile([P, C], f32, tag="gu")
                for k in range(KT):
                    nc.tensor.matmul(
                        psu[:],
                        wg_sb[:, j, 1, k * P : (k + 1) * P],
                        x_sb[:, k, :],
                        start=(k == 0),
                        stop=(k == KT - 1),
                    )
                # glu' = silu(ALPHA*gate + ALPHA*bg) straight from PSUM on the
                # ScalarEngine (gate bias pre-scaled by ALPHA on host).
                # The gate clamp commutes with silu: min(x,L)*sig(A*min(x,L))
                # == min(x*sig(A*x), L*sig(A*L)) since x*sig(A*x) is monotone,
                # so it is applied afterwards, fused into the final multiply.
                glu = work.tile([P, C], bf16, tag="glu")
                nc.scalar.activation(
                    glu[:], psg[:], AF.Silu,
                    bias=b_sb[:, 2 * j : 2 * j + 1], scale=ALPHA,
                )
                # up path is pre-scaled by 1/ALPHA on host:
                # u1 = min(up' + bu', LIMIT/ALPHA); u2 = max(u1, -LIMIT/ALPHA) + 1/ALPHA
                u2 = work.tile([P, C], bf16, tag="u2")
                nc.vector.tensor_scalar(
                    u2[:], psu[:], b_sb[:, 2 * j + 1 : 2 * j + 2], LIMIT / ALPHA,
                    ALU.add, ALU.min,
                )
                nc.vector.tensor_scalar(
                    u2[:], u2[:], -LIMIT / ALPHA, 1.0 / ALPHA, ALU.max, ALU.add
                )
                # act = min(glu', GLUMAX') * u2  ( = (u+1) * min(g,L) * sig(A*min(g,L)) )
                nc.vector.scalar_tensor_tensor(
                    act_sb[:, j, :], glu[:], GLUMAX_S, u2[:], ALU.min, ALU.mult
                )

            # ---- phase 2: y = act^T.T @ W_down ----
            for tm in range(C // P):
                y_t = yout.tile([P, H], bf16, tag="y_sb")
                for n in range(NH):
                    psy = ps_y.tile([P, 512], f32, tag="y")
                    for k in range(IT):
                        nc.tensor.matmul(
                            psy[:],
                            act_sb[:, k, tm * P : (tm + 1) * P],
                            wd_sb[:, n, k * 512 : (k + 1) * 512],
                            start=(k == 0),
                            stop=(k == IT - 1),
                        )
                    nc.vector.tensor_copy(y_t[:, n * 512 : (n + 1) * 512], psy[:])
                nc.sync.dma_start(y[tm * P : (tm + 1) * P, :], y_t[:])

    nc.compile()
    return nc


def get_graph():
    global _GRAPH
    if _GRAPH is None:
        _GRAPH = _build_graph()
    return _GRAPH


def _route(hs, router_w, router_b):
    """Host router: top-2 experts + softmax weights per token (float64)."""
    logits = hs.astype(np.float64) @ router_w.astype(np.float64) + router_b.astype(
        np.float64
    )
    top_idx = np.argsort(-logits, axis=1, kind="stable")[:, :TOP_K]  # [S, K]
    top_vals = np.take_along_axis(logits, top_idx, axis=1)
    ex = np.exp(top_vals - top_vals.max(axis=1, keepdims=True))
    topk_w = ex / ex.sum(axis=1, keepdims=True)  # [S, K]
    scores = np.zeros((hs.shape[0], E), np.float32)
    np.put_along_axis(scores, top_idx, topk_w.astype(np.float32), axis=1)
    return top_idx, topk_w.astype(np.float32), scores


def _dispatch(hs, top_idx, topk_w):
    """Token lists per expert, split into device (first C) and host overflow."""
    dev_idx, dev_w, over_idx, over_w = [], [], [], []
    for e in range(E):
        t, k = np.nonzero(top_idx == e)
        w = topk_w[t, k]
        dev_idx.append(t[:C])
        dev_w.append(w[:C])
        over_idx.append(t[C:])
        over_w.append(w[C:])
    return dev_idx, dev_w, over_idx, over_w


def make_in_maps(hs, gate_up_proj, gate_up_bias, down_proj, dev_idx):
    in_maps = []
    for e in range(E):
        n_e = len(dev_idx[e])
        xt = np.zeros((C, H), np.float32)
        xt[:n_e] = hs[dev_idx[e]]
        xt = np.ascontiguousarray(xt.reshape(C, KT, P).transpose(2, 1, 0)).astype(BF16)

        wg = gate_up_proj[e]  # [H, 2I] interleaved
        # de-interleave to [gate | up]; pre-scale the up half by 1/ALPHA
        wp = np.concatenate([wg[:, 0::2], wg[:, 1::2] * (1.0 / ALPHA)], axis=1)
        wp = (
            wp.reshape(KT, P, MT, P)
            .transpose(2, 1, 0, 3)
            .reshape(MT, P, KT * P)
        )
        # pair-contiguous: wgu[j] = [gate j | up j] k-major blocks
        wp = np.ascontiguousarray(
            np.stack([np.concatenate([wp[j], wp[j + IT]], axis=-1) for j in range(IT)])
        ).astype(BF16)

        wd = np.ascontiguousarray(
            down_proj[e]
            .reshape(IT, P, NH, 512)
            .transpose(1, 2, 0, 3)
            .reshape(P, NH * IT * 512)
        ).astype(BF16)

        bg = gate_up_bias[e]
        bp = np.concatenate([bg[0::2] * ALPHA, bg[1::2] * (1.0 / ALPHA)])
        bp = np.ascontiguousarray(bp.reshape(MT, P).T[:, M_SEQ]).astype(np.float32)

        in_maps.append(
            {
                "xT": np.ascontiguousarray(xt.reshape(P, KT, C)),
                "wgu": np.ascontiguousarray(wp),
                "wdn": np.ascontiguousarray(wd),
                "bgu": bp,
            }
        )
    return in_maps


def _expert_ffn_host(x, wgu_e, bgu_e, wdn_e):
    """Exact fp32 reference math for overflow tokens (no down bias)."""
    gu = x @ wgu_e + bgu_e
    gate = np.minimum(gu[:, 0::2], LIMIT)
    up = np.clip(gu[:, 1::2], -LIMIT, LIMIT)
    with np.errstate(over="ignore"):
        glu = gate / (1.0 + np.exp(np.minimum(-ALPHA * gate, 80.0)))
    return ((up + 1.0) * glu) @ wdn_e


def kernel(
    hidden_states,
    router_w,
    router_b,
    gate_up_proj,
    gate_up_bias,
    down_proj,
    down_bias,
):
    from concourse.bass_utils import run_bass_kernel_spmd

    hs = np.asarray(hidden_states, dtype=np.float32).reshape(S, H)
    router_w = np.asarray(router_w, dtype=np.float32)
    router_b = np.asarray(router_b, dtype=np.float32)
    gate_up_proj = np.asarray(gate_up_proj, dtype=np.float32)
    gate_up_bias = np.asarray(gate_up_bias, dtype=np.float32)
    down_proj = np.asarray(down_proj, dtype=np.float32)
    down_bias = np.asarray(down_bias, dtype=np.float32)

    top_idx, topk_w, scores = _route(hs, router_w, router_b)
    dev_idx, dev_w, over_idx, over_w = _dispatch(hs, top_idx, topk_w)

    nc = get_graph()
    in_maps = make_in_maps(hs, gate_up_proj, gate_up_bias, down_proj, dev_idx)
    res = run_bass_kernel_spmd(nc, in_maps, core_ids=list(range(E)))

    out = np.zeros((S, H), np.float32)
    for e in range(E):
        n_e = len(dev_idx[e])
        if n_e:
            ye = np.asarray(res.results[e]["y"][:n_e], dtype=np.float32) + down_bias[e][None, :]
            out[dev_idx[e]] += dev_w[e][:, None] * ye
        if len(over_idx[e]):
            yo = _expert_ffn_host(
                hs[over_idx[e]], gate_up_proj[e], gate_up_bias[e], down_proj[e]
            ) + down_bias[e][None, :]
            out[over_idx[e]] += over_w[e][:, None] * yo

    return out.reshape(B, S, H), scores.reshape(B, S, E)
